# revision 6
# baseline (speedup 1.0000x reference)
"""BioDecoder teacher-forcing kernel for 8 Trainium2 NeuronCores (Bass/Tile).

v2 — latency-optimized recurrence. Strategy (data-parallel over batch B=8,
one batch element per core):

  - embedding lookup via indirect DMA gather + PE transpose
  - xp0 = W_ih_l0 @ x^T precomputed (+bias b0), stored interleaved with a
    per-slot broadcast of b1 in one "xpb" tile so ONE PE inject matmul
    (N=16) seeds both layers' gate PSUM columns each step
  - the two LSTM layers run wavefront-interleaved with LAG=1, and their
    elementwise work is MERGED: one sigmoid ACT over [128,16] covers both
    layers' gates, one DVE op per cell stage covers both layers ([128,4])
  - layer-1's input projection is folded into the per-step gate matmuls
    (W_ih_l1 @ h1(s) accumulated on the PE alongside W_hh matmuls), so no
    xp1 precompute/chunking exists at all
  - h for both layers is written by a single fused DVE op directly into a
    shared strided history buffer H[128, slot, 4]; the next step's PE
    matmuls read it back as single-column moving operands
  - all transcendentals via Sigmoid only (tanh(x) = 2*sigmoid(2x)-1)
  - gate MLP + output projection (vocab x hidden, fp16) chunked over time,
    logits stored fp16 (host upcasts) to halve the dominant DMA traffic

Self-contained: hardcodes all shapes from the problem spec.
"""

import os
import numpy as np

import concourse.bacc as bacc
import concourse.bass as bass
import concourse.mybir as mybir
import concourse.tile as tile
from concourse.bass import IndirectOffsetOnAxis
from concourse.bass_utils import run_bass_kernel_spmd
from concourse.dve_ops import AFFINE_MUL_REDUCE
from concourse.masks import make_identity

F16 = mybir.dt.float16
F32 = mybir.dt.float32
I32 = mybir.dt.int32
AF = mybir.ActivationFunctionType
OP = mybir.AluOpType

VOCAB, EMBED, HIDDEN = 32000, 128, 256
B, T = 8, 512
TT = T - 1          # 511 recurrence steps
NM = 8              # gate M-tiles (4*HIDDEN / 128)
NK = 2              # hidden K-tiles (HIDDEN / 128)
VN = 512            # vocab tile (one PSUM bank of fp32)
N_CORES = 8

# gate reorder: pytorch i,f,g,o  ->  i,f,o,g (so sigmoid gates are contiguous)
PERM = np.r_[0:256, 256:512, 768:1024, 512:768]


def _col(L, m):
    # PSUM/act column for (layer L, m-tile m): blocks [i|f|o|g] of 4 cols,
    # each block [L0k0, L0k1, L1k0, L1k1] — matching H/c/u/v column order.
    return 4 * (m // 2) + 2 * L + (m % 2)


def _t_chunks(tsteps):
    """Output-projection chunks of up to 128 steps; the final ~32 steps go
    into their own small chunk so the post-recurrence drain is short."""
    out = []
    s = 0
    while s < tsteps:
        e = min(s + 128, tsteps)
        if e == tsteps and e - s > 48:
            e = tsteps - 32
        out.append((s, e))
        s = e
    return out


def build_program(tsteps=TT, dbg=False):
    """Emit the full SPMD program; returns compiled nc."""
    nc = bacc.Bacc("TRN2", target_bir_lowering=False, debug=False,
                   enable_asserts=False, num_devices=N_CORES)

    slots = tsteps + 1
    if dbg:
        hdbg_d = nc.dram_tensor("hdbg", [128, (slots + 1) * 4], F16,
                                kind="ExternalOutput")

    cap_d = nc.dram_tensor("cap", [128, 4], I32, kind="ExternalInput")
    emb_d = nc.dram_tensor("emb", [VOCAB, EMBED], F16, kind="ExternalInput")
    h0_d = nc.dram_tensor("h0", [128, 4], F16, kind="ExternalInput")
    whh0_d = nc.dram_tensor("whh0", [128, 2048], F16, kind="ExternalInput")
    w1_d = nc.dram_tensor("w1", [128, 4096], F16, kind="ExternalInput")
    wih0_d = nc.dram_tensor("wih0", [128, 1024], F16, kind="ExternalInput")
    b0_d = nc.dram_tensor("b0", [128, NM], F32, kind="ExternalInput")
    b1_d = nc.dram_tensor("b1", [128, NM], F32, kind="ExternalInput")
    gw1_d = nc.dram_tensor("gw1", [128, 512], F16, kind="ExternalInput")
    gw2_d = nc.dram_tensor("gw2", [128, NK], F16, kind="ExternalInput")
    gb1_d = nc.dram_tensor("gb1", [128, 2], F32, kind="ExternalInput")
    gb2_d = nc.dram_tensor("gb2", [1, 1], F32, kind="ExternalInput")
    outw_d = nc.dram_tensor("outw", [HIDDEN, VOCAB], F16, kind="ExternalInput")
    logits_d = nc.dram_tensor("logits", [tsteps, VOCAB], F16,
                              kind="ExternalOutput")

    n_gchunks = (tsteps + 127) // 128  # embedding gather chunks

    from contextlib import ExitStack
    with tile.TileContext(nc) as tc, ExitStack() as ctx:
        const = ctx.enter_context(tc.tile_pool(name="const", bufs=1))
        sp = ctx.enter_context(tc.tile_pool(name="sp", bufs=8))
        gp = ctx.enter_context(tc.tile_pool(name="gp", bufs=2))
        lgp = ctx.enter_context(tc.tile_pool(name="lgp", bufs=10))
        pg = ctx.enter_context(tc.tile_pool(name="pg", bufs=2, space="PSUM"))
        pbig = ctx.enter_context(tc.tile_pool(name="pbig", bufs=6, space="PSUM"))

        # ---- persistent SBUF buffers ----
        whh0 = const.tile([128, 2048], F16)
        w1 = const.tile([128, 4096], F16)
        wih0 = const.tile([128, 1024], F16)
        b0 = const.tile([128, NM], F32)
        b1 = const.tile([128, NM], F32)
        gw1 = const.tile([128, 512], F16)
        gw2 = const.tile([128, NK], F16)
        gb1 = const.tile([128, 2], F32)
        gb2 = const.tile([1, 1], F32)
        h0t = const.tile([128, 4], F16)
        idx = const.tile([128, 4], I32)
        ident = const.tile([128, 128], F16)
        ones = const.tile([1, 128], F16)
        zc = const.tile([128, slots], F16)
        xT = const.tile([128, n_gchunks * 128], F16)
        xpb = const.tile([128, slots, 16], F16)
        H = const.tile([128, slots + 1, 4], F16)
        cst = const.tile([128, 4], F32)
        outw = const.tile([128, NK, VOCAB], F16)

        # gather-critical DMAs first: idx gates the embedding gather which
        # gates the whole xp0 pipeline
        for dst, src in ((idx, cap_d), (h0t, h0_d), (wih0, wih0_d),
                         (b0, b0_d), (b1, b1_d), (whh0, whh0_d), (w1, w1_d),
                         (gw1, gw1_d), (gw2, gw2_d), (gb1, gb1_d),
                         (gb2, gb2_d)):
            nc.sync.dma_start(out=dst[:, :], in_=src[:, :])
        # outw: [hidden(2*128), vocab] -> sbuf [128, ki, vocab].  Split into
        # pieces and deprioritized: 45us of DMA that is not needed until the
        # first out-chunk must not starve the startup-critical gather.
        with tc.high_priority(offset=-20000):
            for ki in range(NK):
                for pc in range(8):
                    v0 = pc * 4000
                    nc.sync.dma_start(
                        out=outw[:, ki, v0:v0 + 4000],
                        in_=outw_d[ki * 128:(ki + 1) * 128, v0:v0 + 4000])
        make_identity(nc, ident[:, :])
        nc.vector.memset(ones[:, :], 1.0)
        nc.vector.memset(zc[:, :], 0.0)
        # only the dummy final slot's L0 columns need zeroing — everything
        # else in xpb is covered by the xp0/b1 writes below
        nc.vector.memset(xpb[:, tsteps:slots, :], 0.0)
        nc.vector.memset(cst[:, :], 0.0)
        # initial h (thought) for both layers at slot 0
        nc.vector.tensor_copy(H[:, 0, :], h0t[:, :])

        # ---- embedding gather + transpose ----
        # chunk 0 gates the first xp0 piece (and therefore slot 0); the rest
        # are deprioritized so the scheduler doesn't interleave them ahead
        # of the startup-critical path
        from contextlib import nullcontext
        for j in range(n_gchunks):
            with (nullcontext() if j == 0 else tc.high_priority(offset=-2500)):
                xg = sp.tile([128, 128], F16, tag="xg")
                nc.gpsimd.indirect_dma_start(
                    out=xg[:, :], out_offset=None,
                    in_=emb_d[:, :],
                    in_offset=IndirectOffsetOnAxis(ap=idx[:, j:j + 1], axis=0),
                )
                tp = pbig.tile([128, 512], F16, tag="pb")
                nc.tensor.transpose(tp[:, 0:128], xg[:, :], ident[:, :])
                nc.scalar.copy(xT[:, j * 128:(j + 1) * 128], tp[:, 0:128])

        # ---- xpb: L0 cols = W_ih0 @ x^T + b0; L1 cols = broadcast b1 ----
        # writes split by time-range so slot 0 only waits for the first piece
        xsp = min(128, tsteps)
        for m in range(NM):
            # first-piece matmul only needs the first gather chunk, so the
            # recurrence can start while the rest of the gather streams in
            ps = pbig.tile([128, 512], F32, tag="pb")
            nc.tensor.matmul(ps[:, 0:xsp], wih0[:, m * 128:(m + 1) * 128],
                             xT[:, 0:xsp], start=True, stop=True)
            nc.scalar.activation(xpb[:, 0:xsp, _col(0, m)], ps[:, 0:xsp],
                                 AF.Identity, bias=b0[:, m:m + 1])
            nc.vector.tensor_scalar_add(xpb[:, 0:xsp, _col(1, m)],
                                        zc[:, 0:xsp], b1[:, m:m + 1])
        for m in range(NM):
            if tsteps > xsp:
                ps = pbig.tile([128, 512], F32, tag="pb")
                nc.tensor.matmul(ps[:, 0:tsteps - xsp],
                                 wih0[:, m * 128:(m + 1) * 128],
                                 xT[:, xsp:tsteps], start=True, stop=True)
                nc.scalar.activation(xpb[:, xsp:tsteps, _col(0, m)],
                                     ps[:, 0:tsteps - xsp],
                                     AF.Identity, bias=b0[:, m:m + 1])
            nc.vector.tensor_scalar_add(xpb[:, xsp:slots, _col(1, m)],
                                        zc[:, xsp:slots], b1[:, m:m + 1])

        # ---- output-projection chunk (reads h2 from H[:, s+2, 2:4]) ----
        def out_chunk(ts_, te_, wide=False):
            nt = te_ - ts_
            # t1 = sig(2*(H2 @ gw1.T + gb1))  (tanh folded into gw2/gb2)
            t1 = gp.tile([128, NK, 128], F16, tag="t1")
            for mi in range(2):
                ps = pbig.tile([128, 512], F32, tag="pb")
                for ki in range(NK):
                    nc.tensor.matmul(
                        ps[:, 0:nt],
                        gw1[:, ki * 256 + mi * 128: ki * 256 + (mi + 1) * 128],
                        H[:, ts_ + 2:te_ + 2, 2 + ki],
                        start=(ki == 0), stop=(ki == NK - 1))
                nc.scalar.activation(t1[:, mi, 0:nt], ps[:, 0:nt], AF.Sigmoid,
                                     bias=gb1[:, mi:mi + 1], scale=2.0)
            psg = pbig.tile([128, 512], F32, tag="pb")
            for ki in range(NK):
                nc.tensor.matmul(psg[0:1, 0:nt], gw2[:, ki:ki + 1],
                                 t1[:, ki, 0:nt],
                                 start=(ki == 0), stop=(ki == NK - 1))
            g16 = gp.tile([1, 128], F16, tag="g16")
            nc.scalar.activation(g16[0:1, 0:nt], psg[0:1, 0:nt], AF.Sigmoid,
                                 bias=gb2[0:1, 0:1])
            bc = pbig.tile([128, 512], F32, tag="pb")
            nc.tensor.matmul(bc[:, 0:nt], ones[0:1, :], g16[0:1, 0:nt],
                             start=True, stop=True)
            gated = gp.tile([128, NK, 128], F16, tag="gated")
            for ki in range(NK):
                nc.vector.tensor_mul(gated[:, ki, 0:nt],
                                     H[:, ts_ + 2:te_ + 2, 2 + ki],
                                     bc[:, 0:nt])
            # logits: alternate the PSUM->SBUF(+fp16 cast) copy between the
            # DVE and ACT engines so neither hosts all of it
            nvt = (VOCAB + VN - 1) // VN
            for vt in range(nvt):
                v0 = vt * VN
                nv = min(VN, VOCAB - v0)
                ps = pbig.tile([128, 512], F32, tag="pb")
                # halves keep PE busy-quanta small so recurrence gate
                # matmuls interleave with at most ~210ns of delay
                for hv in range(2):
                    h0_, h1_ = hv * 256, min((hv + 1) * 256, nv)
                    if h0_ >= nv:
                        break
                    for ki in range(NK):
                        nc.tensor.matmul(ps[0:nt, h0_:h1_], gated[:, ki, 0:nt],
                                         outw[:, ki, v0 + h0_:v0 + h1_],
                                         start=(ki == 0), stop=(ki == NK - 1))
                lg = lgp.tile([128, 512], F16, tag="lg")
                if wide:
                    # tail: throughput matters, nothing to contend with
                    if vt % 2 == 0:
                        nc.vector.tensor_copy(lg[0:nt, 0:nv], ps[0:nt, 0:nv])
                    else:
                        nc.scalar.copy(lg[0:nt, 0:nv], ps[0:nt, 0:nv])
                else:
                    # overlapped with the recurrence: quarter-width copies,
                    # alternating DVE/ACT, so a copy never blocks a chain op
                    # for more than ~170ns of engine time
                    for q in range(4):
                        qs, qe = q * 128, min((q + 1) * 128, nv)
                        if qs >= nv:
                            break
                        if (vt + q) % 2 == 0:
                            nc.vector.tensor_copy(lg[0:nt, qs:qe],
                                                  ps[0:nt, qs:qe])
                        else:
                            nc.scalar.copy(lg[0:nt, qs:qe], ps[0:nt, qs:qe])
                # round-robin the issuing engine so multiple HWDGE queues
                # run the logits DMAs in parallel (Pool is otherwise idle;
                # ACT has slack only in the tail)
                dma_eng = (nc.gpsimd, nc.sync)[vt % 2]
                dma_eng.dma_start(out=logits_d[ts_:te_, v0:v0 + nv],
                                  in_=lg[0:nt, 0:nv])

        tch = _t_chunks(tsteps)
        tci = {te: (ts_, te) for ts_, te in tch}

        # ---- merged 2-layer wavefront, one slot per L0 step ----
        for t in range(slots):
            g_ps = pg.tile([128, 16], F32, tag="g")
            # seed all 16 gate columns: [xp0(t)+b0 | b1] via one inject
            nc.tensor.matmul(g_ps[:, 0:16], ident[:, :], xpb[:, t, 0:16],
                             start=True, stop=False)
            # L1 recurrent part first: W_hh1 @ h2(s-1) reads H cols 2:4,
            # written one slot earlier — these 16 matmuls stream through the
            # PE while the previous slot's cell update is still in flight
            for m in range(NM):
                co = _col(1, m)
                for j in (2, 3):
                    nc.tensor.matmul(
                        g_ps[:, co:co + 1],
                        w1[:, j * 1024 + m * 128: j * 1024 + (m + 1) * 128],
                        H[:, t, j:j + 1],
                        start=False, stop=False)
            # L0: gates += W_hh0 @ h1(t-1)   (H cols 0:2, just written)
            for m in range(NM):
                co = _col(0, m)
                for ki in range(NK):
                    nc.tensor.matmul(
                        g_ps[:, co:co + 1],
                        whh0[:, ki * 1024 + m * 128: ki * 1024 + (m + 1) * 128],
                        H[:, t, ki:ki + 1],
                        start=False, stop=(ki == NK - 1))
            # L1 input part: W_ih1 @ h1(s)   (H cols 0:2, just written)
            for m in range(NM):
                co = _col(1, m)
                for j in (0, 1):
                    nc.tensor.matmul(
                        g_ps[:, co:co + 1],
                        w1[:, j * 1024 + m * 128: j * 1024 + (m + 1) * 128],
                        H[:, t, j:j + 1],
                        start=False, stop=(j == 1))
            # one sigmoid for every gate of both layers
            a = sp.tile([128, 16], F32, tag="a")
            nc.scalar.activation(a[:, :], g_ps[:, :], AF.Sigmoid)
            # u = sig_i * tanh(g) = (2*sig(2g) - 1) * sig_i
            u = sp.tile([128, 4], F32, tag="u")
            nc.vector._custom_dve(AFFINE_MUL_REDUCE, out=u[:, :],
                                  in0=a[:, 12:16], in1=a[:, 0:4],
                                  s0=2.0, s1=-1.0)
            # v = sig_f * c_prev ; c = u + v
            v = sp.tile([128, 4], F32, tag="v")
            nc.vector.tensor_mul(v[:, :], a[:, 4:8], cst[:, :])
            nc.vector.tensor_add(cst[:, :], u[:, :], v[:, :])
            # sc = sig(2c) ; h = (2*sc - 1) * sig_o  -> H[:, t+1, :]
            sc = sp.tile([128, 4], F32, tag="sc")
            nc.scalar.activation(sc[:, :], cst[:, :], AF.Sigmoid, scale=2.0)
            nc.vector._custom_dve(AFFINE_MUL_REDUCE, out=H[:, t + 1, :],
                                  in0=sc[:, :], in1=a[:, 8:12],
                                  s0=2.0, s1=-1.0)
            if t == 0:
                # slot 0's L1 half-step was a throwaway (s=-1): restore its
                # state to (h=thought, c=0) before slot 1 consumes it
                nc.vector.tensor_copy(H[:, 1, 2:4], h0t[:, 2:4])
                nc.vector.memset(cst[:, 2:4], 0.0)
            if t in tci:
                ts_, te_ = tci[t]
                with tc.high_priority(offset=-12000):
                    out_chunk(ts_, te_, wide=(te_ == tsteps))

        if dbg:
            nc.sync.dma_start(out=hdbg_d[:, :], in_=H[:, :, :])

    nc.compile()
    return nc


def prep_inputs(inputs, tsteps=TT):
    """Host-side: permute/tile/cast weights, build per-core in_maps."""
    g = {k: np.asarray(v) for k, v in inputs.items()}

    def f16(x):
        return np.ascontiguousarray(x.astype(np.float16))

    def gate_scale(wp):
        # pre-scale the g-gate block (post-perm rows 768:1024) by 2 so that
        # sigmoid(pre) directly yields sig(2g) for the tanh identity
        wp = wp.copy()
        wp[768:1024] *= 2.0
        return wp

    def tile_whh(w):  # [1024, 256] -> [128, ki*1024 + m*128 + j]
        wp = gate_scale(w[PERM].astype(np.float32))
        return f16(wp.reshape(8, 128, 2, 128).transpose(3, 2, 0, 1)
                   .reshape(128, 2048))

    def tile_wih0(w):  # [1024, 128] -> [128(e), m*128 + j]
        wp = gate_scale(w[PERM].astype(np.float32))
        return f16(wp.reshape(8, 128, 128).transpose(2, 0, 1).reshape(128, 1024))

    whh0 = tile_whh(g["w_hh_l0"])
    wih0 = tile_wih0(g["w_ih_l0"])
    # layer-1 combined: [wih1_k0 | wih1_k1 | whh1_k0 | whh1_k1] chunks of 1024
    w1cat = np.concatenate(
        [tile_whh(g["w_ih_l1"]), tile_whh(g["w_hh_l1"])], axis=1)

    bp0 = gate_scale((g["b_ih_l0"] + g["b_hh_l0"])[PERM].astype(np.float32))
    bp1 = gate_scale((g["b_ih_l1"] + g["b_hh_l1"])[PERM].astype(np.float32))
    b0 = np.ascontiguousarray(bp0.reshape(8, 128).T)   # [128, m]
    b1 = np.ascontiguousarray(bp1.reshape(8, 128).T)

    gw1 = f16(g["gate_w1"].astype(np.float32).reshape(2, 128, 2, 128)
              .transpose(3, 2, 0, 1).reshape(128, 512))
    # t1 is stored as sigmoid(2x); tanh = 2*t1-1 folded into gw2/gb2:
    #   gate pre-act = gw2 @ (2*t1-1) + gb2 = (2*gw2) @ t1 + (gb2 - sum(gw2))
    gw2v = g["gate_w2"].astype(np.float32).reshape(256)
    gw2 = f16((2.0 * gw2v).reshape(2, 128).T)
    gb2 = np.array([[g["gate_b2"].astype(np.float32).reshape(()) - gw2v.sum()]],
                   dtype=np.float32)
    gb1 = np.ascontiguousarray(
        (2.0 * g["gate_b1"].astype(np.float32)).reshape(2, 128).T)

    emb = f16(g["emb_w"])
    outw = f16(g["out_w"].astype(np.float32).T)       # [256, 32000]

    caps = np.asarray(g["captions"], dtype=np.int32)  # [B, T]
    thought = g["thought"].astype(np.float32)          # [B, 256]

    n_gchunks = (tsteps + 127) // 128
    in_maps = []
    for b in range(B):
        capb = np.zeros((128, 4), dtype=np.int32)
        toks = caps[b, :tsteps]
        for j in range(n_gchunks):
            seg = toks[j * 128:(j + 1) * 128]
            capb[:len(seg), j] = seg
        th = thought[b].reshape(2, 128).T.astype(np.float16)  # [128, k]
        h0 = np.ascontiguousarray(np.concatenate([th, th], axis=1))  # [128,4]
        in_maps.append({
            "cap": capb, "emb": emb, "h0": h0,
            "whh0": whh0, "w1": w1cat, "wih0": wih0,
            "b0": b0, "b1": b1, "gw1": gw1, "gw2": gw2,
            "gb1": gb1, "gb2": gb2, "outw": outw,
        })
    return in_maps


_cached = {}


def _get_program(tsteps=TT):
    if tsteps not in _cached:
        _cached[tsteps] = build_program(tsteps)
    return _cached[tsteps]


def kernel(**inputs) -> np.ndarray:
    tsteps = int(os.environ.get("BIODEC_T", TT))
    nc = _get_program(tsteps)
    in_maps = prep_inputs(inputs, tsteps)
    res = run_bass_kernel_spmd(nc, in_maps, list(range(N_CORES)))
    out = np.stack([res.results[b]["logits"] for b in range(B)], axis=0)
    out = out.astype(np.float32)
    out_b = np.asarray(inputs["out_b"], dtype=np.float32)
    if np.any(out_b):
        out = out + out_b
    return out


# revision 7
# speedup vs baseline: 5.2195x; 5.2195x over previous
"""BioDecoder teacher-forcing kernel for 8 Trainium2 NeuronCores (Bass/Tile).

v2 — latency-optimized recurrence. Strategy (data-parallel over batch B=8,
one batch element per core):

  - embedding lookup via indirect DMA gather + PE transpose
  - xp0 = W_ih_l0 @ x^T precomputed (+bias b0), stored interleaved with a
    per-slot broadcast of b1 in one "xpb" tile so ONE PE inject matmul
    (N=16) seeds both layers' gate PSUM columns each step
  - the two LSTM layers run wavefront-interleaved with LAG=1, and their
    elementwise work is MERGED: one sigmoid ACT over [128,16] covers both
    layers' gates, one DVE op per cell stage covers both layers ([128,4])
  - layer-1's input projection is folded into the per-step gate matmuls
    (W_ih_l1 @ h1(s) accumulated on the PE alongside W_hh matmuls), so no
    xp1 precompute/chunking exists at all
  - h for both layers is written by a single fused DVE op directly into a
    shared strided history buffer H[128, slot, 4]; the next step's PE
    matmuls read it back as single-column moving operands
  - all transcendentals via Sigmoid only (tanh(x) = 2*sigmoid(2x)-1)
  - gate MLP + output projection (vocab x hidden, fp16) chunked over time,
    logits stored fp16 (host upcasts) to halve the dominant DMA traffic

Self-contained: hardcodes all shapes from the problem spec.
"""

import os
import numpy as np

import concourse.bacc as bacc
import concourse.bass as bass
import concourse.mybir as mybir
import concourse.tile as tile
from concourse.bass import IndirectOffsetOnAxis
from concourse.bass_utils import run_bass_kernel_spmd
from concourse.dve_ops import AFFINE_MUL_REDUCE
from concourse.masks import make_identity

F16 = mybir.dt.float16
F32 = mybir.dt.float32
I32 = mybir.dt.int32
AF = mybir.ActivationFunctionType
OP = mybir.AluOpType

VOCAB, EMBED, HIDDEN = 32000, 128, 256
B, T = 8, 512
TT = T - 1          # 511 recurrence steps
NM = 8              # gate M-tiles (4*HIDDEN / 128)
NK = 2              # hidden K-tiles (HIDDEN / 128)
VN = 512            # vocab tile (one PSUM bank of fp32)
N_CORES = 8

# gate reorder: pytorch i,f,g,o  ->  i,f,o,g (so sigmoid gates are contiguous)
PERM = np.r_[0:256, 256:512, 768:1024, 512:768]


def _col(L, m):
    # PSUM/act column for (layer L, m-tile m): blocks [i|f|o|g] of 4 cols,
    # each block [L0k0, L0k1, L1k0, L1k1] — matching H/c/u/v column order.
    return 4 * (m // 2) + 2 * L + (m % 2)


def _t_chunks(tsteps):
    """Output-projection chunks of up to 128 steps; the final ~32 steps go
    into their own small chunk so the post-recurrence drain is short."""
    out = []
    s = 0
    while s < tsteps:
        e = min(s + 128, tsteps)
        if e == tsteps and e - s > 48:
            e = tsteps - 32
        out.append((s, e))
        s = e
    return out


def build_program(tsteps=TT, dbg=False):
    """Emit the full SPMD program; returns compiled nc."""
    nc = bacc.Bacc("TRN2", target_bir_lowering=False, debug=False,
                   enable_asserts=False, num_devices=N_CORES)

    slots = tsteps + 1
    if dbg:
        hdbg_d = nc.dram_tensor("hdbg", [128, (slots + 1) * 4], F16,
                                kind="ExternalOutput")

    cap_d = nc.dram_tensor("cap", [128, 4], I32, kind="ExternalInput")
    emb_d = nc.dram_tensor("emb", [VOCAB, EMBED], F16, kind="ExternalInput")
    h0_d = nc.dram_tensor("h0", [128, 4], F16, kind="ExternalInput")
    whh0_d = nc.dram_tensor("whh0", [128, 2048], F16, kind="ExternalInput")
    w1_d = nc.dram_tensor("w1", [128, 4096], F16, kind="ExternalInput")
    wih0_d = nc.dram_tensor("wih0", [128, 1024], F16, kind="ExternalInput")
    b0_d = nc.dram_tensor("b0", [128, NM], F32, kind="ExternalInput")
    b1_d = nc.dram_tensor("b1", [128, NM], F32, kind="ExternalInput")
    gw1_d = nc.dram_tensor("gw1", [128, 512], F16, kind="ExternalInput")
    gw2_d = nc.dram_tensor("gw2", [128, NK], F16, kind="ExternalInput")
    gb1_d = nc.dram_tensor("gb1", [128, 2], F32, kind="ExternalInput")
    gb2_d = nc.dram_tensor("gb2", [1, 1], F32, kind="ExternalInput")
    outw_d = nc.dram_tensor("outw", [HIDDEN, VOCAB], F16, kind="ExternalInput")
    logits_d = nc.dram_tensor("logits", [tsteps, VOCAB], F16,
                              kind="ExternalOutput")

    n_gchunks = (tsteps + 127) // 128  # embedding gather chunks

    from contextlib import ExitStack
    with tile.TileContext(nc) as tc, ExitStack() as ctx:
        const = ctx.enter_context(tc.tile_pool(name="const", bufs=1))
        sp = ctx.enter_context(tc.tile_pool(name="sp", bufs=8))
        gp = ctx.enter_context(tc.tile_pool(name="gp", bufs=2))
        lgp = ctx.enter_context(tc.tile_pool(name="lgp", bufs=10))
        pg = ctx.enter_context(tc.tile_pool(name="pg", bufs=2, space="PSUM"))
        pbig = ctx.enter_context(tc.tile_pool(name="pbig", bufs=6, space="PSUM"))

        # ---- persistent SBUF buffers ----
        whh0 = const.tile([128, 2048], F16)
        w1 = const.tile([128, 4096], F16)
        wih0 = const.tile([128, 1024], F16)
        b0 = const.tile([128, NM], F32)
        b1 = const.tile([128, NM], F32)
        gw1 = const.tile([128, 512], F16)
        gw2 = const.tile([128, NK], F16)
        gb1 = const.tile([128, 2], F32)
        gb2 = const.tile([1, 1], F32)
        h0t = const.tile([128, 4], F16)
        idx = const.tile([128, 4], I32)
        ident = const.tile([128, 128], F16)
        ones = const.tile([1, 128], F16)
        zc = const.tile([128, slots], F16)
        xT = const.tile([128, n_gchunks * 128], F16)
        xpb = const.tile([128, slots, 16], F16)
        H = const.tile([128, slots + 1, 4], F16)
        cst = const.tile([128, 4], F32)
        outw = const.tile([128, NK, VOCAB], F16)

        # gather-critical DMAs first: idx gates the embedding gather which
        # gates the whole xp0 pipeline
        for dst, src in ((idx, cap_d), (h0t, h0_d), (wih0, wih0_d),
                         (b0, b0_d), (b1, b1_d), (whh0, whh0_d), (w1, w1_d),
                         (gw1, gw1_d), (gw2, gw2_d), (gb1, gb1_d),
                         (gb2, gb2_d)):
            nc.sync.dma_start(out=dst[:, :], in_=src[:, :])
        # outw: [hidden(2*128), vocab] -> sbuf [128, ki, vocab].  Split into
        # pieces and deprioritized: 45us of DMA that is not needed until the
        # first out-chunk must not starve the startup-critical gather.
        with tc.high_priority(offset=-20000):
            for ki in range(NK):
                for pc in range(8):
                    v0 = pc * 4000
                    nc.sync.dma_start(
                        out=outw[:, ki, v0:v0 + 4000],
                        in_=outw_d[ki * 128:(ki + 1) * 128, v0:v0 + 4000])
        make_identity(nc, ident[:, :])
        nc.vector.memset(ones[:, :], 1.0)
        nc.vector.memset(zc[:, :], 0.0)
        # only the dummy final slot's L0 columns need zeroing — everything
        # else in xpb is covered by the xp0/b1 writes below
        nc.vector.memset(xpb[:, tsteps:slots, :], 0.0)
        nc.vector.memset(cst[:, :], 0.0)
        # initial h (thought) for both layers at slot 0
        nc.vector.tensor_copy(H[:, 0, :], h0t[:, :])

        # ---- embedding gather + transpose ----
        # chunk 0 gates the first xp0 piece (and therefore slot 0); the rest
        # are deprioritized so the scheduler doesn't interleave them ahead
        # of the startup-critical path
        from contextlib import nullcontext
        for j in range(n_gchunks):
            with (nullcontext() if j == 0 else tc.high_priority(offset=-2500)):
                xg = sp.tile([128, 128], F16, tag="xg")
                nc.gpsimd.indirect_dma_start(
                    out=xg[:, :], out_offset=None,
                    in_=emb_d[:, :],
                    in_offset=IndirectOffsetOnAxis(ap=idx[:, j:j + 1], axis=0),
                )
                tp = pbig.tile([128, 512], F16, tag="pb")
                nc.tensor.transpose(tp[:, 0:128], xg[:, :], ident[:, :])
                nc.scalar.copy(xT[:, j * 128:(j + 1) * 128], tp[:, 0:128])

        # ---- xpb: L0 cols = W_ih0 @ x^T + b0; L1 cols = broadcast b1 ----
        # writes split by time-range so slot 0 only waits for the first piece
        xsp = min(128, tsteps)
        for m in range(NM):
            # first-piece matmul only needs the first gather chunk, so the
            # recurrence can start while the rest of the gather streams in
            ps = pbig.tile([128, 512], F32, tag="pb")
            nc.tensor.matmul(ps[:, 0:xsp], wih0[:, m * 128:(m + 1) * 128],
                             xT[:, 0:xsp], start=True, stop=True)
            nc.scalar.activation(xpb[:, 0:xsp, _col(0, m)], ps[:, 0:xsp],
                                 AF.Identity, bias=b0[:, m:m + 1])
            nc.vector.tensor_scalar_add(xpb[:, 0:xsp, _col(1, m)],
                                        zc[:, 0:xsp], b1[:, m:m + 1])
        for m in range(NM):
            if tsteps > xsp:
                ps = pbig.tile([128, 512], F32, tag="pb")
                nc.tensor.matmul(ps[:, 0:tsteps - xsp],
                                 wih0[:, m * 128:(m + 1) * 128],
                                 xT[:, xsp:tsteps], start=True, stop=True)
                nc.scalar.activation(xpb[:, xsp:tsteps, _col(0, m)],
                                     ps[:, 0:tsteps - xsp],
                                     AF.Identity, bias=b0[:, m:m + 1])
            nc.vector.tensor_scalar_add(xpb[:, xsp:slots, _col(1, m)],
                                        zc[:, xsp:slots], b1[:, m:m + 1])

        # ---- output-projection chunk (reads h2 from H[:, s+2, 2:4]) ----
        def out_chunk(ts_, te_, wide=False):
            nt = te_ - ts_
            # t1 = sig(2*(H2 @ gw1.T + gb1))  (tanh folded into gw2/gb2)
            t1 = gp.tile([128, NK, 128], F16, tag="t1")
            for mi in range(2):
                ps = pbig.tile([128, 512], F32, tag="pb")
                for ki in range(NK):
                    nc.tensor.matmul(
                        ps[:, 0:nt],
                        gw1[:, ki * 256 + mi * 128: ki * 256 + (mi + 1) * 128],
                        H[:, ts_ + 2:te_ + 2, 2 + ki],
                        start=(ki == 0), stop=(ki == NK - 1))
                nc.scalar.activation(t1[:, mi, 0:nt], ps[:, 0:nt], AF.Sigmoid,
                                     bias=gb1[:, mi:mi + 1], scale=2.0)
            psg = pbig.tile([128, 512], F32, tag="pb")
            for ki in range(NK):
                nc.tensor.matmul(psg[0:1, 0:nt], gw2[:, ki:ki + 1],
                                 t1[:, ki, 0:nt],
                                 start=(ki == 0), stop=(ki == NK - 1))
            g16 = gp.tile([1, 128], F16, tag="g16")
            nc.scalar.activation(g16[0:1, 0:nt], psg[0:1, 0:nt], AF.Sigmoid,
                                 bias=gb2[0:1, 0:1])
            bc = pbig.tile([128, 512], F32, tag="pb")
            nc.tensor.matmul(bc[:, 0:nt], ones[0:1, :], g16[0:1, 0:nt],
                             start=True, stop=True)
            gated = gp.tile([128, NK, 128], F16, tag="gated")
            for ki in range(NK):
                nc.vector.tensor_mul(gated[:, ki, 0:nt],
                                     H[:, ts_ + 2:te_ + 2, 2 + ki],
                                     bc[:, 0:nt])
            # logits: alternate the PSUM->SBUF(+fp16 cast) copy between the
            # DVE and ACT engines so neither hosts all of it.  Two vocab
            # tiles share one staging buffer so each DMA moves 2KB/partition;
            # DMAs round-robin two HWDGE queues (SP + Pool-SWDGE).
            nvt = (VOCAB + VN - 1) // VN
            pair = 2 if wide else 1
            for vt0 in range(0, nvt, pair):
                lg = lgp.tile([128, 1024], F16, tag="lg")
                lg_w = 0
                for vt in range(vt0, min(vt0 + pair, nvt)):
                    v0 = vt * VN
                    nv = min(VN, VOCAB - v0)
                    o = (vt - vt0) * VN
                    ps = pbig.tile([128, 512], F32, tag="pb")
                    # halves keep PE busy-quanta small so recurrence gate
                    # matmuls interleave with at most ~210ns of delay
                    for hv in range(2):
                        h0_, h1_ = hv * 256, min((hv + 1) * 256, nv)
                        if h0_ >= nv:
                            break
                        for ki in range(NK):
                            nc.tensor.matmul(ps[0:nt, h0_:h1_],
                                             gated[:, ki, 0:nt],
                                             outw[:, ki, v0 + h0_:v0 + h1_],
                                             start=(ki == 0),
                                             stop=(ki == NK - 1))
                    if wide:
                        # tail: throughput matters, nothing to contend with
                        if vt % 2 == 0:
                            nc.vector.tensor_copy(lg[0:nt, o:o + nv],
                                                  ps[0:nt, 0:nv])
                        else:
                            nc.scalar.copy(lg[0:nt, o:o + nv], ps[0:nt, 0:nv])
                    else:
                        # overlapped with the recurrence: quarter-width
                        # copies, alternating DVE/ACT, so a copy never blocks
                        # a chain op for more than ~170ns of engine time
                        for q in range(4):
                            qs, qe = q * 128, min((q + 1) * 128, nv)
                            if qs >= nv:
                                break
                            if (vt + q) % 2 == 0:
                                nc.vector.tensor_copy(lg[0:nt, o + qs:o + qe],
                                                      ps[0:nt, qs:qe])
                            else:
                                nc.scalar.copy(lg[0:nt, o + qs:o + qe],
                                               ps[0:nt, qs:qe])
                    lg_w += nv
                dma_eng = (nc.gpsimd, nc.sync)[(vt0 // 2) % 2]
                dma_eng.dma_start(
                    out=logits_d[ts_:te_, vt0 * VN:vt0 * VN + lg_w],
                    in_=lg[0:nt, 0:lg_w])

        tch = _t_chunks(tsteps)
        tci = {te: (ts_, te) for ts_, te in tch}

        # ---- merged 2-layer wavefront, one slot per L0 step ----
        for t in range(slots):
            g_ps = pg.tile([128, 16], F32, tag="g")
            # seed all 16 gate columns: [xp0(t)+b0 | b1] via one inject
            nc.tensor.matmul(g_ps[:, 0:16], ident[:, :], xpb[:, t, 0:16],
                             start=True, stop=False)
            # L1 recurrent part first: W_hh1 @ h2(s-1) reads H cols 2:4,
            # written one slot earlier — these 16 matmuls stream through the
            # PE while the previous slot's cell update is still in flight
            for m in range(NM):
                co = _col(1, m)
                for j in (2, 3):
                    nc.tensor.matmul(
                        g_ps[:, co:co + 1],
                        w1[:, j * 1024 + m * 128: j * 1024 + (m + 1) * 128],
                        H[:, t, j:j + 1],
                        start=False, stop=False)
            # L0: gates += W_hh0 @ h1(t-1)   (H cols 0:2, just written)
            for m in range(NM):
                co = _col(0, m)
                for ki in range(NK):
                    nc.tensor.matmul(
                        g_ps[:, co:co + 1],
                        whh0[:, ki * 1024 + m * 128: ki * 1024 + (m + 1) * 128],
                        H[:, t, ki:ki + 1],
                        start=False, stop=(ki == NK - 1))
            # L1 input part: W_ih1 @ h1(s)   (H cols 0:2, just written)
            for m in range(NM):
                co = _col(1, m)
                for j in (0, 1):
                    nc.tensor.matmul(
                        g_ps[:, co:co + 1],
                        w1[:, j * 1024 + m * 128: j * 1024 + (m + 1) * 128],
                        H[:, t, j:j + 1],
                        start=False, stop=(j == 1))
            # one sigmoid for every gate of both layers
            a = sp.tile([128, 16], F32, tag="a")
            nc.scalar.activation(a[:, :], g_ps[:, :], AF.Sigmoid)
            # u = sig_i * tanh(g) = (2*sig(2g) - 1) * sig_i
            u = sp.tile([128, 4], F32, tag="u")
            nc.vector._custom_dve(AFFINE_MUL_REDUCE, out=u[:, :],
                                  in0=a[:, 12:16], in1=a[:, 0:4],
                                  s0=2.0, s1=-1.0)
            # v = sig_f * c_prev ; c = u + v
            v = sp.tile([128, 4], F32, tag="v")
            nc.vector.tensor_mul(v[:, :], a[:, 4:8], cst[:, :])
            nc.vector.tensor_add(cst[:, :], u[:, :], v[:, :])
            # sc = sig(2c) ; h = (2*sc - 1) * sig_o  -> H[:, t+1, :]
            sc = sp.tile([128, 4], F32, tag="sc")
            nc.scalar.activation(sc[:, :], cst[:, :], AF.Sigmoid, scale=2.0)
            nc.vector._custom_dve(AFFINE_MUL_REDUCE, out=H[:, t + 1, :],
                                  in0=sc[:, :], in1=a[:, 8:12],
                                  s0=2.0, s1=-1.0)
            if t == 0:
                # slot 0's L1 half-step was a throwaway (s=-1): restore its
                # state to (h=thought, c=0) before slot 1 consumes it
                nc.vector.tensor_copy(H[:, 1, 2:4], h0t[:, 2:4])
                nc.vector.memset(cst[:, 2:4], 0.0)
            if t in tci:
                ts_, te_ = tci[t]
                with tc.high_priority(offset=-12000):
                    out_chunk(ts_, te_, wide=(te_ == tsteps))

        if dbg:
            nc.sync.dma_start(out=hdbg_d[:, :], in_=H[:, :, :])

    nc.compile()
    return nc


def prep_inputs(inputs, tsteps=TT):
    """Host-side: permute/tile/cast weights, build per-core in_maps."""
    g = {k: np.asarray(v) for k, v in inputs.items()}

    def f16(x):
        return np.ascontiguousarray(x.astype(np.float16))

    def gate_scale(wp):
        # pre-scale the g-gate block (post-perm rows 768:1024) by 2 so that
        # sigmoid(pre) directly yields sig(2g) for the tanh identity
        wp = wp.copy()
        wp[768:1024] *= 2.0
        return wp

    def tile_whh(w):  # [1024, 256] -> [128, ki*1024 + m*128 + j]
        wp = gate_scale(w[PERM].astype(np.float32))
        return f16(wp.reshape(8, 128, 2, 128).transpose(3, 2, 0, 1)
                   .reshape(128, 2048))

    def tile_wih0(w):  # [1024, 128] -> [128(e), m*128 + j]
        wp = gate_scale(w[PERM].astype(np.float32))
        return f16(wp.reshape(8, 128, 128).transpose(2, 0, 1).reshape(128, 1024))

    whh0 = tile_whh(g["w_hh_l0"])
    wih0 = tile_wih0(g["w_ih_l0"])
    # layer-1 combined: [wih1_k0 | wih1_k1 | whh1_k0 | whh1_k1] chunks of 1024
    w1cat = np.concatenate(
        [tile_whh(g["w_ih_l1"]), tile_whh(g["w_hh_l1"])], axis=1)

    bp0 = gate_scale((g["b_ih_l0"] + g["b_hh_l0"])[PERM].astype(np.float32))
    bp1 = gate_scale((g["b_ih_l1"] + g["b_hh_l1"])[PERM].astype(np.float32))
    b0 = np.ascontiguousarray(bp0.reshape(8, 128).T)   # [128, m]
    b1 = np.ascontiguousarray(bp1.reshape(8, 128).T)

    gw1 = f16(g["gate_w1"].astype(np.float32).reshape(2, 128, 2, 128)
              .transpose(3, 2, 0, 1).reshape(128, 512))
    # t1 is stored as sigmoid(2x); tanh = 2*t1-1 folded into gw2/gb2:
    #   gate pre-act = gw2 @ (2*t1-1) + gb2 = (2*gw2) @ t1 + (gb2 - sum(gw2))
    gw2v = g["gate_w2"].astype(np.float32).reshape(256)
    gw2 = f16((2.0 * gw2v).reshape(2, 128).T)
    gb2 = np.array([[g["gate_b2"].astype(np.float32).reshape(()) - gw2v.sum()]],
                   dtype=np.float32)
    gb1 = np.ascontiguousarray(
        (2.0 * g["gate_b1"].astype(np.float32)).reshape(2, 128).T)

    emb = f16(g["emb_w"])
    outw = f16(g["out_w"].astype(np.float32).T)       # [256, 32000]

    caps = np.asarray(g["captions"], dtype=np.int32)  # [B, T]
    thought = g["thought"].astype(np.float32)          # [B, 256]

    n_gchunks = (tsteps + 127) // 128
    in_maps = []
    for b in range(B):
        capb = np.zeros((128, 4), dtype=np.int32)
        toks = caps[b, :tsteps]
        for j in range(n_gchunks):
            seg = toks[j * 128:(j + 1) * 128]
            capb[:len(seg), j] = seg
        th = thought[b].reshape(2, 128).T.astype(np.float16)  # [128, k]
        h0 = np.ascontiguousarray(np.concatenate([th, th], axis=1))  # [128,4]
        in_maps.append({
            "cap": capb, "emb": emb, "h0": h0,
            "whh0": whh0, "w1": w1cat, "wih0": wih0,
            "b0": b0, "b1": b1, "gw1": gw1, "gw2": gw2,
            "gb1": gb1, "gb2": gb2, "outw": outw,
        })
    return in_maps


_cached = {}


def _get_program(tsteps=TT):
    if tsteps not in _cached:
        _cached[tsteps] = build_program(tsteps)
    return _cached[tsteps]


def kernel(**inputs) -> np.ndarray:
    tsteps = int(os.environ.get("BIODEC_T", TT))
    nc = _get_program(tsteps)
    in_maps = prep_inputs(inputs, tsteps)
    res = run_bass_kernel_spmd(nc, in_maps, list(range(N_CORES)))
    out = np.stack([res.results[b]["logits"] for b in range(B)], axis=0)
    out = out.astype(np.float32)
    out_b = np.asarray(inputs["out_b"], dtype=np.float32)
    if np.any(out_b):
        out = out + out_b
    return out
